# revision 1
# baseline (speedup 1.0000x reference)
"""Diagonal-Gaussian NLL loss on 8 Trainium2 NeuronCores.

loss = -mean_i log N(y_i; mu_i, diag(sigma_i))
     = 0.5 * (NT*log(2*pi) + (sum_ij ln(sigma_ij) + sum_ij (y-mu)_ij^2/sigma_ij) / BS)

Data-parallel over the batch dim: each core processes 512 rows of each
(4096, 8192) fp32 tensor (48 MB of HBM reads per core -> memory bound).

Per (128 x FD) chunk on-chip:
  ACT:  l  = Ln(sigma)          [accum_out -> per-partition logdet partial]
        r  = Exp(-l) = 1/sigma  (ACT Reciprocal is banned for accuracy; Ln/Exp/
                                 Square all live in one activation table set)
        d2 = Square(d)
  DVE:  d  = y - mu
        q  = d2 * r             [tensor_tensor_reduce: accum_out -> quad partial]

Per-core output is two (128, NCH) fp32 partial-sum tiles; the host sums them
in fp64 and applies the constant/scale. No collectives needed.
"""

import math

import numpy as np

BS, NT = 4096, 8192
NCORES = 8
ROWS = BS // NCORES  # 512 rows per core
P = 128              # SBUF partitions
PT = ROWS // P       # 4 partition tiles
FD = 4096            # free-dim chunk size (2 MB DMAs; q shares l's slot to fit SBUF)
FC = NT // FD        # free chunks per row-tile
NCH = PT * FC        # chunks per core

_CACHE = {}


def _build_nc(repeats=1):
    import concourse.bacc as bacc
    import concourse.mybir as mybir
    import concourse.tile as tile

    f32 = mybir.dt.float32
    bf16 = mybir.dt.bfloat16
    AF = mybir.ActivationFunctionType
    OP = mybir.AluOpType

    nc = bacc.Bacc("TRN2", target_bir_lowering=False, debug=False,
                   num_devices=NCORES)
    mu = nc.dram_tensor("mu", [ROWS, NT], f32, kind="ExternalInput").ap()
    sg = nc.dram_tensor("sigma", [ROWS, NT], f32, kind="ExternalInput").ap()
    ty = nc.dram_tensor("target_y", [ROWS, NT], f32, kind="ExternalInput").ap()
    ncols = NCH * repeats
    lsum_d = nc.dram_tensor("lsum", [P, ncols], f32, kind="ExternalOutput").ap()
    qsum_d = nc.dram_tensor("qsum", [P, ncols], f32, kind="ExternalOutput").ap()

    with tile.TileContext(nc) as tc:
        with tc.tile_pool(name="io", bufs=2) as io_pool, \
             tc.tile_pool(name="mid", bufs=2) as mid_pool, \
             tc.tile_pool(name="acc", bufs=1) as acc_pool:
            lsum = acc_pool.tile([P, ncols], f32)
            qsum = acc_pool.tile([P, ncols], f32)
            for rep in range(repeats):
              ch = rep * NCH
              for pt in range(PT):
                rows = slice(pt * P, (pt + 1) * P)
                for fc in range(FC):
                    cols = slice(fc * FD, (fc + 1) * FD)
                    sg_t = io_pool.tile([P, FD], f32, tag="sg")
                    nc.sync.dma_start(sg_t[:], sg[rows, cols])
                    y_t = io_pool.tile([P, FD], f32, tag="y")
                    nc.sync.dma_start(y_t[:], ty[rows, cols])
                    mu_t = io_pool.tile([P, FD], f32, tag="mu")
                    nc.sync.dma_start(mu_t[:], mu[rows, cols])

                    # NB: ACT ops with 16-bit input AND output have a
                    # systematic +1.2e-4 bias on this HW — keep l/d fp32 so
                    # every activation has an fp32 side.
                    l_t = mid_pool.tile([P, FD], f32, tag="l")
                    nc.scalar.activation(l_t[:], sg_t[:], AF.Ln,
                                         accum_out=lsum[:, ch:ch + 1])
                    r_t = mid_pool.tile([P, FD], bf16, tag="r")
                    nc.scalar.activation(r_t[:], l_t[:], AF.Exp, scale=-1.0)
                    d_t = mid_pool.tile([P, FD], f32, tag="d")
                    nc.vector.tensor_tensor(d_t[:], y_t[:], mu_t[:], OP.subtract)
                    d2_t = mid_pool.tile([P, FD], bf16, tag="d2")
                    nc.scalar.activation(d2_t[:], d_t[:], AF.Square)
                    q_t = mid_pool.tile([P, FD], bf16, tag="l")
                    nc.vector.scalar_tensor_tensor(
                        q_t[:], d2_t[:], 1.0, r_t[:], OP.mult, OP.mult,
                        accum_out=qsum[:, ch:ch + 1])
                    ch += 1
            nc.sync.dma_start(lsum_d[:], lsum[:])
            nc.sync.dma_start(qsum_d[:], qsum[:])
    nc.compile()
    return nc


def _run(inputs, trace=False):
    from concourse.bass_utils import run_bass_kernel_spmd

    if "nc" not in _CACHE:
        _CACHE["nc"] = _build_nc()
    nc = _CACHE["nc"]

    mu = np.ascontiguousarray(inputs["mu"], dtype=np.float32)
    sg = np.ascontiguousarray(inputs["sigma"], dtype=np.float32)
    ty = np.ascontiguousarray(inputs["target_y"], dtype=np.float32)

    in_maps = [
        {
            "mu": mu[c * ROWS:(c + 1) * ROWS],
            "sigma": sg[c * ROWS:(c + 1) * ROWS],
            "target_y": ty[c * ROWS:(c + 1) * ROWS],
        }
        for c in range(NCORES)
    ]
    res = run_bass_kernel_spmd(nc, in_maps, list(range(NCORES)), trace=trace)

    total = 0.0
    for core_out in res.results:
        total += core_out["lsum"].astype(np.float64).sum()
        total += core_out["qsum"].astype(np.float64).sum()
    loss = 0.5 * (NT * math.log(2.0 * math.pi) + total / BS)
    return np.asarray(loss, dtype=np.float32), res


def kernel(**inputs):
    out, _ = _run(inputs)
    return out



# revision 2
# speedup vs baseline: 1.8865x; 1.8865x over previous
"""Diagonal-Gaussian NLL loss on 8 Trainium2 NeuronCores — v2.

loss = 0.5 * (NT*log(2*pi) + (sum_ij ln(sigma_ij) + sum_ij (y-mu)_ij^2/sigma_ij) / BS)

Data-parallel over batch: each core reads 512 rows of each tensor.
HBM traffic cut via input quantization (tolerance is 2e-2; bias ~1e-3):
  y, mu  -> bf16   (8 MiB/core each)
  sigma  -> fp8e4  (4 MiB/core)       total 20 MiB/core vs 48 fp32

Per (128 x FD) chunk:
  ACT:  l = Ln(sigma)  [accum_out -> logdet partial; Ln+Exp share one table set]
        r = Exp(-l) = 1/sigma
  DVE:  d = y - mu; d2 = d*d; q = d2*r      (three bf16 tensor_tensor, 2x mode;
        accumulating DVE ops run 1x, so the reduction moves to...)
  PE:   ones[128,1]^T @ q[:, j*512:(j+1)*512] accumulated into one PSUM bank
        (TensorE is otherwise idle; partition-sum + free-dim-block accumulate)

Outputs per core: lsum [128, NCH] (Ln accums) + qsum [1, 512] (PSUM row);
host sums in fp64 and applies the constant/scale.
"""

import math

import numpy as np

BS, NT = 4096, 8192
NCORES = 8
ROWS = BS // NCORES  # 512
P = 128
PT = ROWS // P       # 4
FD = 4096
FC = NT // FD
NCH = PT * FC        # chunks per core
QB = 512             # PSUM bank block (fp32)

L_DTYPE = "bf16"
SIGMA_DTYPE = "fp8"
Y_DTYPE = "bf16"
IO_BUFS = 3
MID_BUFS = 2
Q_PATH = "mm"

_CACHE = {}


def _dt(mybir, name):
    return {
        "bf16": mybir.dt.bfloat16,
        "fp8": mybir.dt.float8e4,
        "f32": mybir.dt.float32,
    }[name]


def _build_nc(repeats=1):
    import concourse.bacc as bacc
    import concourse.mybir as mybir
    import concourse.tile as tile

    f32 = mybir.dt.float32
    bf16 = mybir.dt.bfloat16
    AF = mybir.ActivationFunctionType
    OP = mybir.AluOpType
    sdt = _dt(mybir, SIGMA_DTYPE)
    ldt = _dt(mybir, L_DTYPE)
    ydt = _dt(mybir, Y_DTYPE)

    nc = bacc.Bacc("TRN2", target_bir_lowering=False, debug=False,
                   num_devices=NCORES)
    mu = nc.dram_tensor("mu", [ROWS, NT], ydt, kind="ExternalInput").ap()
    sg = nc.dram_tensor("sigma", [ROWS, NT], sdt, kind="ExternalInput").ap()
    ty = nc.dram_tensor("target_y", [ROWS, NT], ydt, kind="ExternalInput").ap()
    ncols = NCH * repeats
    lsum_d = nc.dram_tensor("lsum", [P, ncols], f32, kind="ExternalOutput").ap()
    qshape = [1, QB] if Q_PATH == "mm" else [P, ncols]
    qsum_d = nc.dram_tensor("qsum", qshape, f32, kind="ExternalOutput").ap()

    n_mm = repeats * NCH * (FD // QB)
    with tile.TileContext(nc) as tc:
        with tc.tile_pool(name="io", bufs=IO_BUFS) as io_pool, \
             tc.tile_pool(name="mid", bufs=MID_BUFS) as mid_pool, \
             tc.tile_pool(name="acc", bufs=1) as acc_pool, \
             tc.psum_pool(name="ps", bufs=1) as ps_pool:
            ones = acc_pool.tile([P, 1], bf16)
            if Q_PATH == "mm":
                nc.any.memset(ones[:], 1.0)
            lsum = acc_pool.tile([P, ncols], f32)
            qsum = acc_pool.tile([P, ncols], f32)
            psq = ps_pool.tile([1, QB], f32)
            mm = 0
            for rep in range(repeats):
                for pt in range(PT):
                    rows = slice(pt * P, (pt + 1) * P)
                    for fc in range(FC):
                        col = rep * NCH + pt * FC + fc
                        cols = slice(fc * FD, (fc + 1) * FD)
                        sg_t = io_pool.tile([P, FD], sdt, tag="sg")
                        nc.sync.dma_start(sg_t[:], sg[rows, cols])
                        y_t = io_pool.tile([P, FD], ydt, tag="y")
                        nc.sync.dma_start(y_t[:], ty[rows, cols])
                        mu_t = io_pool.tile([P, FD], ydt, tag="mu")
                        nc.sync.dma_start(mu_t[:], mu[rows, cols])

                        l_t = mid_pool.tile([P, FD], ldt, tag="l")
                        nc.scalar.activation(l_t[:], sg_t[:], AF.Ln,
                                             accum_out=lsum[:, col:col + 1])
                        r_t = mid_pool.tile([P, FD], bf16, tag="r")
                        nc.scalar.activation(r_t[:], l_t[:], AF.Exp, scale=-1.0)
                        d_t = mid_pool.tile([P, FD], bf16, tag="d")
                        nc.vector.tensor_tensor(d_t[:], y_t[:], mu_t[:],
                                                OP.subtract)
                        d2_t = mid_pool.tile([P, FD], bf16, tag="d2")
                        nc.vector.tensor_tensor(d2_t[:], d_t[:], d_t[:],
                                                OP.mult)
                        if Q_PATH == "mm":
                            q_t = mid_pool.tile([P, FD], bf16, tag="q")
                            nc.vector.tensor_tensor(q_t[:], d2_t[:], r_t[:],
                                                    OP.mult)
                            for j in range(FD // QB):
                                nc.tensor.matmul(
                                    psq[:], ones[:],
                                    q_t[:, j * QB:(j + 1) * QB],
                                    start=(mm == 0), stop=(mm == n_mm - 1))
                                mm += 1
                        else:
                            q_t = mid_pool.tile([P, FD], bf16, tag="q")
                            nc.vector.scalar_tensor_tensor(
                                q_t[:], d2_t[:], 1.0, r_t[:], OP.mult,
                                OP.mult, accum_out=qsum[:, col:col + 1])
            if Q_PATH == "mm":
                qs = acc_pool.tile([1, QB], f32)
                nc.vector.tensor_copy(qs[:], psq[:])
                nc.sync.dma_start(qsum_d[:], qs[:])
            else:
                nc.sync.dma_start(qsum_d[:], qsum[:])
            nc.sync.dma_start(lsum_d[:], lsum[:])
    nc.compile()
    return nc


def _convert(inputs):
    import ml_dtypes

    f8 = np.dtype(ml_dtypes.float8_e4m3)
    bf = np.dtype(ml_dtypes.bfloat16)
    ynp = {"bf16": bf, "fp8": f8}[Y_DTYPE]
    snp = {"bf16": bf, "fp8": f8}[SIGMA_DTYPE]
    mu = np.ascontiguousarray(inputs["mu"]).astype(ynp)
    ty = np.ascontiguousarray(inputs["target_y"]).astype(ynp)
    sg = np.ascontiguousarray(inputs["sigma"]).astype(snp)
    return mu, sg, ty


def make_in_maps(inputs):
    mu, sg, ty = _convert(inputs)
    return [
        {
            "mu": mu[c * ROWS:(c + 1) * ROWS],
            "sigma": sg[c * ROWS:(c + 1) * ROWS],
            "target_y": ty[c * ROWS:(c + 1) * ROWS],
        }
        for c in range(NCORES)
    ]


def _run(inputs, trace=False):
    from concourse.bass_utils import run_bass_kernel_spmd

    if "nc" not in _CACHE:
        _CACHE["nc"] = _build_nc()
    nc = _CACHE["nc"]

    in_maps = make_in_maps(inputs)
    res = run_bass_kernel_spmd(nc, in_maps, list(range(NCORES)), trace=trace)

    total = 0.0
    for core_out in res.results:
        total += core_out["lsum"].astype(np.float64).sum()
        total += core_out["qsum"].astype(np.float64).sum()
    loss = 0.5 * (NT * math.log(2.0 * math.pi) + total / BS)
    return np.asarray(loss, dtype=np.float32), res


def kernel(**inputs):
    out, _ = _run(inputs)
    return out
